# revision 13
# baseline (speedup 1.0000x reference)
"""Trainium2 Bass kernel for nn_GCNBranch_neg (6-layer GCN, shared graph).

Sharding: nodes across 8 cores; edges partitioned by destination (scatter side
core-local); per layer the scaled features h' = dinv * (x@Wg) are AllGathered
as bf16 node rows, each core dma_gathers its edges' source rows from HBM and
segment-sums them on the TensorEngine with one-hot fp8 chunk matrices (S)
built once on-device. norm = dinv[src]*dinv[dst] is folded into h' (src side)
and a per-column multiply post-aggregation (dst side). GCNConv input matmuls
are folded on the host: (x@W + b)@Wg = x@(W Wg) + b@Wg.
"""
import os
import sys

sys.path.insert(0, "/opt/trn_rl_repo")

SKIP_GATHER = bool(int(os.environ.get("GCN_SKIP_GATHER", "0")))
SKIP_SEG = bool(int(os.environ.get("GCN_SKIP_SEG", "0")))
SKIP_AG = bool(int(os.environ.get("GCN_SKIP_AG", "0")))

import numpy as np
import ml_dtypes

import concourse.bass as bass
import concourse.tile as tile
from concourse import bacc, mybir
from concourse import bass_utils
from concourse.library_config import mlp as mlp_lib

dt = mybir.dt
F32, BF16, I32, I16, FP8 = dt.float32, dt.bfloat16, dt.int32, dt.int16, dt.float8e4
AF = mybir.ActivationFunctionType
ALU = mybir.AluOpType


class Cfg:
    def __init__(self, N=50000, E=400000, IN_DIM=512, H1=256, H2=62, OUT=128,
                 lo_lim=None):
        self.N, self.E = N, E
        self.NC = 8
        self.IN_DIM, self.H1, self.H2, self.OUT = IN_DIM, H1, H2, OUT
        self.H2P = 128
        assert N % self.NC == 0
        self.NLOC = N // self.NC
        self.NWIN = (self.NLOC + 127) // 128
        self.NLOCP = self.NWIN * 128
        self.NG = self.NC * self.NLOCP
        self.LO_LIM = lo_lim if lo_lim is not None else min(32768, self.NG)
        self.HI_BASE = max(0, self.NG - self.LO_LIM)
        self.GRP = 7
        self.MAXP = 7  # number of gcn passes to emit (bisection knob)
        self.NSLAB_IN = IN_DIM // 128
        self.NSLAB_H1 = H1 // 128


def _pad(a, shape):
    out = np.zeros(shape, np.float64)
    out[tuple(slice(0, s) for s in a.shape)] = a
    return out


def _wtile(w):
    k, m = w.shape
    assert k % 128 == 0
    return np.ascontiguousarray(
        np.asarray(w).reshape(k // 128, 128, m).transpose(1, 0, 2)
    ).reshape(128, -1).astype(ml_dtypes.bfloat16)


def _btile(b):
    m = b.shape[0]
    assert m % 128 == 0
    return np.ascontiguousarray(
        np.asarray(b).reshape(m // 128, 128).T).astype(np.float32)


def preprocess(cfg, inputs):
    c = cfg
    ei = np.asarray(inputs["edge_index_neg"]).astype(np.int64)
    src = np.concatenate([ei[0], np.arange(c.N, dtype=np.int64)])
    dst = np.concatenate([ei[1], np.arange(c.N, dtype=np.int64)])
    deg = np.bincount(dst, minlength=c.N).astype(np.float32)

    g_src = (src // c.NLOC) * c.NLOCP + (src % c.NLOC)
    is_lo = g_src < c.LO_LIM
    core = dst // c.NLOC
    dloc = dst - core * c.NLOC
    win = dloc // 128
    rel = dloc - win * 128

    order = np.lexsort((g_src, win, core))
    coreo, wino, relo, go, loo = (core[order], win[order], rel[order],
                                  g_src[order], is_lo[order])
    per = {}
    nlo = np.zeros((c.NC, c.NWIN), np.int64)
    nhi = np.zeros((c.NC, c.NWIN), np.int64)
    for cc in range(c.NC):
        m = coreo == cc
        per[cc] = (wino[m], relo[m], go[m], loo[m])
        nlo[cc] = np.bincount(per[cc][0][per[cc][3]], minlength=c.NWIN)
        nhi[cc] = np.bincount(per[cc][0][~per[cc][3]], minlength=c.NWIN)

    nch_lo = np.ceil(nlo.max(0) / 128).astype(np.int64)
    nch_hi = np.ceil(nhi.max(0) / 128).astype(np.int64)
    nch_hi = np.maximum(nch_hi, 1)
    nch_lo = np.maximum(nch_lo, 1)
    clo_start = np.concatenate([[0], np.cumsum(nch_lo)])
    chi_start = np.concatenate([[0], np.cumsum(nch_hi)])
    CLO, CHI = int(clo_start[-1]), int(chi_start[-1])

    zrow_lo = c.NLOC                                      # rank-0 pad row (zeros)
    zrow_hi = (c.NC - 1) * c.NLOCP + c.NLOC - c.HI_BASE   # last-rank pad row

    sched = dict(nch_lo=nch_lo, nch_hi=nch_hi, clo_start=clo_start,
                 chi_start=chi_start, CLO=CLO, CHI=CHI)

    per_core = []
    for cc in range(c.NC):
        wins, rels, gs, los = per[cc]
        idx_lo = np.full(CLO * 128, zrow_lo, np.int64)
        idx_hi = np.full(CHI * 128, zrow_hi, np.int64)
        dst_rel = np.full((CLO + CHI) * 128, -1, np.int32)
        for half in (True, False):
            hm = los == half
            w_h, r_h, g_h = wins[hm], rels[hm], gs[hm]
            starts = clo_start if half else chi_start
            idx_arr = idx_lo if half else idx_hi
            base_off = 0 if half else CLO * 128
            g_adj = g_h if half else g_h - c.HI_BASE
            cnt = np.bincount(w_h, minlength=c.NWIN)
            ends = np.cumsum(cnt)
            pos_in_w = np.arange(len(w_h)) - (ends[w_h] - cnt[w_h])
            slots = starts[w_h] * 128 + pos_in_w
            idx_arr[slots] = g_adj
            dst_rel[base_off + slots] = r_h
        assert idx_lo.max() < 32768 and idx_hi.max() < 32768
        assert idx_lo.min() >= 0 and idx_hi.min() >= 0

        def wrap(a):
            return np.tile(a.astype(np.int16).reshape(-1, 16).T, (8, 1)).copy()

        deg_loc = np.ones(c.NLOCP, np.float32)
        deg_loc[:c.NLOC] = deg[cc * c.NLOC:(cc + 1) * c.NLOC]
        x = np.asarray(inputs["x"])[cc * c.NLOC:(cc + 1) * c.NLOC]
        per_core.append(dict(
            x_t=np.ascontiguousarray(np.asarray(x, np.float32).T).astype(
                ml_dtypes.bfloat16),
            idx_lo=wrap(idx_lo), idx_hi=wrap(idx_hi),
            dst_rel=np.ascontiguousarray(
                dst_rel.reshape(CLO + CHI, 128).T).astype(np.int32),
            deg=np.ascontiguousarray(
                deg_loc.reshape(c.NWIN, 128).T).astype(np.float32),
        ))

    f = lambda k: np.asarray(inputs[k]).astype(np.float64)
    W1, b1, W2, b2, W3, b3 = f("W1"), f("b1"), f("W2"), f("b2"), f("W3"), f("b3")
    Wg = [f(f"Wg{i}") for i in range(1, 7)]
    bg = [f(f"bg{i}") for i in range(1, 7)]
    W2p = _pad(W2, (c.H1, c.H2P))
    b2p = _pad(b2, (c.H2P,))
    Wg2p = _pad(Wg[1], (c.H2P, c.H2P))
    bg2p = _pad(bg[1], (c.H2P,))
    W3p = _pad(W3, (c.H2P, c.OUT))

    weights = dict(
        w1=_wtile(W1), w2=_wtile(W2p), w3=_wtile(W3p),
        w1g=_wtile(W1 @ Wg[0]), w2g=_wtile(W2p @ Wg2p), w3g=_wtile(W3p @ Wg[2]),
        wg4=_wtile(Wg[3]), wg5=_wtile(Wg[4]), wg6=_wtile(Wg[5]),
        b1=_btile(b1), b2=_btile(b2p), b3=_btile(b3),
        b1g=_btile(b1 @ Wg[0]), b2g=_btile(b2p @ Wg2p), b3g=_btile(b3 @ Wg[2]),
        bg1=_btile(bg[0]), bg2=_btile(bg2p), bg3=_btile(0.5 * bg[2]),
        bg4=_btile(0.5 * bg[3]), bg5=_btile(0.25 * bg[4]),
        bg6=_btile(0.25 * bg[5]),
    )
    in_maps = []
    for cc in range(c.NC):
        m = dict(per_core[cc])
        m.update(weights)
        in_maps.append(m)
    return sched, in_maps


def build_nc(cfg, sched):
    c = cfg
    CLO, CHI = sched["CLO"], sched["CHI"]
    TOT = CLO + CHI
    nch_lo, nch_hi = sched["nch_lo"], sched["nch_hi"]
    clo_start, chi_start = sched["clo_start"], sched["chi_start"]
    NV = c.NLOC
    NS1 = c.NSLAB_H1

    nc = bacc.Bacc(None, debug=False, num_devices=c.NC,
                   num_swdge_queues=int(os.environ.get("GCN_NQ", "2")),
                   dynamic_dma_scratch_size=int(os.environ.get("GCN_DMA_SCRATCH", "16384")))

    def din(name, shape, d):
        return nc.dram_tensor(name, shape, d, kind="ExternalInput")

    x_t = din("x_t", [c.IN_DIM, c.NLOC], BF16)
    idx_lo_d = din("idx_lo", [128, CLO * 8], I16)
    idx_hi_d = din("idx_hi", [128, CHI * 8], I16)
    dst_rel_d = din("dst_rel", [128, TOT], I32)
    deg_d = din("deg", [128, c.NWIN], F32)
    wshape = dict(w1=(c.NSLAB_IN, c.H1), w2=(NS1, c.H2P), w3=(1, c.OUT),
                  w1g=(c.NSLAB_IN, c.H1), w2g=(NS1, c.H2P), w3g=(1, c.OUT),
                  wg4=(1, c.OUT), wg5=(1, c.OUT), wg6=(1, c.OUT))
    wd = {k: din(k, [128, ns * m], BF16) for k, (ns, m) in wshape.items()}
    bshape = dict(b1=NS1, b2=1, b3=1, b1g=NS1, b2g=1, b3g=1,
                  bg1=NS1, bg2=1, bg3=1, bg4=1, bg5=1, bg6=1)
    bd = {k: din(k, [128, n], F32) for k, n in bshape.items()}
    y_t = nc.dram_tensor("y_t", [c.OUT, c.NLOC], F32, kind="ExternalOutput")

    ag_in = [nc.dram_tensor(f"ag_in{i}", [c.NLOCP, 128], BF16) for i in range(2)]
    ag_out = [nc.dram_tensor(f"ag_out{i}", [c.NG, 128], BF16, addr_space="Shared")
              for i in range(2)]
    dv_d = nc.dram_tensor("dv_d", [1, c.NLOCP], F32)

    dram_m = lambda n: nc.dram_tensor(n, [128, NV], F32)
    dram_mb = lambda n: nc.dram_tensor(n, [128, NV], BF16)
    m_x1l = [dram_m("m_x1l_a"), dram_m("m_x1l_b")][:NS1]
    m_x2l, m_x3l = dram_m("m_x2l"), dram_m("m_x3l")
    m_x1 = [dram_m("m_x1_a"), dram_m("m_x1_b")][:NS1]
    m_x2, m_x3, m_x4, m_x5 = [dram_m(f"m_x{i}") for i in range(2, 6)]
    mb_x1 = [dram_mb("mb_x1_a"), dram_mb("mb_x1_b")][:NS1]
    mb_x2, mb_x3, mb_x4, mb_x5 = [dram_mb(f"mb_x{i}") for i in range(2, 6)]

    nc.gpsimd.load_library(mlp_lib)

    with tile.TileContext(nc) as tc:
      with tc.tile_pool(name="const", bufs=1) as const:
        iota_t = const.tile([128, 128], I32)
        nc.gpsimd.iota(iota_t[:], pattern=[[1, 128]], channel_multiplier=0)
        iota_p = const.tile([128, 1], I32)
        nc.gpsimd.iota(iota_p[:], pattern=[[0, 1]], channel_multiplier=1)
        ident = const.tile([128, 128], BF16)
        nc.vector.tensor_tensor(out=ident[:], in0=iota_t[:],
                                in1=iota_p[:].broadcast_to([128, 128]),
                                op=ALU.is_equal)

        idx_lo_t = const.tile([128, CLO * 8], I16)
        nc.sync.dma_start(idx_lo_t[:], idx_lo_d[:, :])
        idx_hi_t = const.tile([128, CHI * 8], I16)
        nc.sync.dma_start(idx_hi_t[:], idx_hi_d[:, :])

        wt = {}
        for k, (ns, m) in wshape.items():
            wt[k] = const.tile([128, ns, m], BF16, name=f"w_{k}", tag=f"w_{k}")
            nc.sync.dma_start(
                wt[k][:], wd[k].ap().rearrange("p (s m) -> p s m", s=ns))
        bt = {}
        for k, n in bshape.items():
            bt[k] = const.tile([128, n], F32, name=f"b_{k}", tag=f"b_{k}")
            nc.sync.dma_start(bt[k][:], bd[k][:, :])

        dst_t = const.tile([128, TOT], I32)
        nc.sync.dma_start(dst_t[:], dst_rel_d[:, :])
        S_t = const.tile([128, TOT, 128], FP8)
        step = (TOT + 3) // 4
        for a in range(0, TOT, step):
            b = min(TOT, a + step)
            nc.vector.tensor_tensor(
                out=S_t[:, a:b, :],
                in0=dst_t[:, a:b].broadcast_to([128, b - a, 128]),
                in1=iota_t[:].unsqueeze(1).broadcast_to([128, b - a, 128]),
                op=ALU.is_equal)

        dinv_bc = const.tile([128, c.NLOCP], F32)
        with tc.tile_pool(name="dvtmp", bufs=1) as dvp:
            deg_t = dvp.tile([128, c.NWIN], F32)
            nc.sync.dma_start(deg_t[:], deg_d[:, :])
            sq_t = dvp.tile([128, c.NWIN], F32)
            nc.scalar.sqrt(sq_t[:], deg_t[:])
            dv_t = dvp.tile([128, c.NWIN], F32)
            nc.vector.reciprocal(dv_t[:], sq_t[:])
            nc.sync.dma_start(
                dv_d.ap().rearrange("a (t p) -> (a p) t", p=128), dv_t[:])
            dv_row = dvp.tile([1, c.NLOCP], F32)
            nc.sync.dma_start(dv_row[:], dv_d[0:1, :])
            nc.gpsimd.partition_broadcast(dinv_bc[:], dv_row[:])

        # ----- helpers -----
        def dense_pass(xbf, xoff, c0, c1, wkey, n_in, n_out, evict, psd, dstg):
            for s in range(n_out):
                for cc0 in range(c0, c1, 512):
                    cw = min(512, c1 - cc0)
                    ps = psd.tile([128, 512], F32, tag="psd")
                    for fi in range(n_in):
                        nc.tensor.matmul(
                            ps[:, :cw],
                            wt[wkey][:, fi, s * 128:(s + 1) * 128],
                            xbf[:, fi, cc0 - xoff:cc0 - xoff + cw],
                            start=(fi == 0), stop=(fi == n_in - 1))
                    evict(s, cc0, cw, ps, dstg)

        def ev_master(bkey, masters):
            def ev(s, cc0, cw, ps, dstg):
                st = dstg.tile([128, 512], F32, tag="evst")
                nc.vector.tensor_scalar(
                    st[:, :cw], ps[:, :cw], bt[bkey][:, s:s + 1], None, ALU.add)
                nc.sync.dma_start(masters[s][:, cc0:cc0 + cw], st[:, :cw])
            return ev

        def ev_h(hT, bkey):
            def ev(s, cc0, cw, ps, dstg):
                sc = bt[bkey][:, s:s + 1] if bkey else 0.0
                nc.vector.scalar_tensor_tensor(
                    out=hT[:, s, cc0:cc0 + cw], in0=ps[:, :cw], scalar=sc,
                    in1=dinv_bc[:, cc0:cc0 + cw], op0=ALU.add, op1=ALU.mult)
            return ev

        def transpose_store(hT, s, pass_idx):
            agi = ag_in[pass_idx % 2]
            with tc.tile_pool(name="hrows", bufs=2) as hrp, \
                 tc.tile_pool(name="pst", bufs=2, space="PSUM") as pst:
                hr = hrp.tile([128, c.NWIN, 128], BF16, tag="hr")
                for t in range(c.NWIN):
                    tp = pst.tile([128, 128], BF16, tag="tp")
                    nc.tensor.transpose(
                        tp[:], hT[:, s, t * 128:(t + 1) * 128], ident[:])
                    nc.scalar.activation(hr[:, t, :], tp[:], AF.Copy)
                nc.sync.dma_start(
                    agi.ap().rearrange("(t p) f -> p t f", p=128), hr[:])

        def allgather(pass_idx):
            if SKIP_AG:
                return
            nc.gpsimd.collective_compute(
                "AllGather", ALU.bypass,
                replica_groups=[list(range(c.NC))],
                ins=[ag_in[pass_idx % 2].ap().opt()],
                outs=[ag_out[pass_idx % 2].ap().opt()])

        def _gathers(mlo, mhi, ago, cl0, cl1, ch0, ch1):
            sp = bool(int(os.environ.get("GCN_SINGLE_PACKET", "0")))
            nc.gpsimd.dma_gather(
                mlo[:, :cl1 - cl0, :], ago[:, :],
                idx_lo_t[:, cl0 * 8:cl1 * 8],
                (cl1 - cl0) * 128, (cl1 - cl0) * 128, 128, queue_num=0,
                single_packet=sp)
            nc.gpsimd.dma_gather(
                mhi[:, :ch1 - ch0, :], ago[c.HI_BASE:, :],
                idx_hi_t[:, ch0 * 8:ch1 * 8],
                (ch1 - ch0) * 128, (ch1 - ch0) * 128, 128,
                queue_num=int(os.environ.get("GCN_QHI", "1")),
                single_packet=sp)

        def gcn_combine(pp, m_in, m_out, mb_out, bg_tile, bg_col, s_scale, relu):
            ago = ag_out[pp % 2]
            ngrp = (c.NWIN + c.GRP - 1) // c.GRP
            max_clo = max(int(clo_start[min(c.NWIN, (g + 1) * c.GRP)]
                              - clo_start[g * c.GRP]) for g in range(ngrp))
            max_chi = max(int(chi_start[min(c.NWIN, (g + 1) * c.GRP)]
                              - chi_start[g * c.GRP]) for g in range(ngrp))
            with tc.tile_pool(name="mlo", bufs=2) as plo, \
                 tc.tile_pool(name="mhi", bufs=2) as phi, \
                 tc.tile_pool(name="stg", bufs=2) as stg, \
                 tc.tile_pool(name="pseg", bufs=2, space="PSUM") as pseg:
                for g in range(ngrp):
                    w0, w1 = g * c.GRP, min(c.NWIN, (g + 1) * c.GRP)
                    nw = w1 - w0
                    cl0, cl1 = int(clo_start[w0]), int(clo_start[w1])
                    ch0, ch1 = int(chi_start[w0]), int(chi_start[w1])
                    mlo = plo.tile([128, max_clo, 128], BF16, tag="mlo")
                    mhi = phi.tile([128, max_chi, 128], BF16, tag="mhi")
                    if SKIP_GATHER:
                        nc.vector.memset(mlo[:], 0.0)
                        nc.vector.memset(mhi[:], 0.0)
                    else:
                        _gathers(mlo, mhi, ago, cl0, cl1, ch0, ch1)
                    _unused = lambda: nc.gpsimd.dma_gather(
                        mlo[:, :cl1 - cl0, :], ago[:, :],
                        idx_lo_t[:, cl0 * 8:cl1 * 8],
                        (cl1 - cl0) * 128, (cl1 - cl0) * 128, 128, queue_num=0)
                    pss = [pseg.tile([128, 512], F32, name=f"ps{i}", tag=f"ps{i}")
                           for i in range((nw + 3) // 4)]
                    for wl in range(nw):
                        w = w0 + wl
                        ps = pss[wl // 4]
                        pc = (wl % 4) * 128
                        ntot = int(nch_lo[w]) + int(nch_hi[w])
                        if SKIP_SEG:
                            nc.tensor.matmul(
                                ps[:, pc:pc + 128], mlo[:, 0, :], S_t[:, 0, :],
                                start=True, stop=True)
                            continue
                        ki = 0
                        for k in range(int(nch_lo[w])):
                            nc.tensor.matmul(
                                ps[:, pc:pc + 128],
                                mlo[:, int(clo_start[w]) - cl0 + k, :],
                                S_t[:, int(clo_start[w]) + k, :],
                                start=(ki == 0), stop=(ki == ntot - 1))
                            ki += 1
                        for k in range(int(nch_hi[w])):
                            nc.tensor.matmul(
                                ps[:, pc:pc + 128],
                                mhi[:, int(chi_start[w]) - ch0 + k, :],
                                S_t[:, CLO + int(chi_start[w]) + k, :],
                                start=(ki == 0), stop=(ki == ntot - 1))
                            ki += 1
                    gc0 = w0 * 128
                    gcw = min(NV, w1 * 128) - gc0
                    t1 = stg.tile([128, c.GRP * 128], F32, tag="t1")
                    for pi, ps in enumerate(pss):
                        a = pi * 512
                        b = min(gcw, a + 512)
                        if b <= a:
                            break
                        nc.vector.tensor_tensor(
                            out=t1[:, a:b], in0=ps[:, :b - a],
                            in1=dinv_bc[:, gc0 + a:gc0 + b], op=ALU.mult)
                    t2 = stg.tile([128, c.GRP * 128], F32, tag="t2")
                    nc.scalar.activation(
                        t2[:, :gcw], t1[:, :gcw],
                        AF.Relu if relu else AF.Identity,
                        bias=bg_tile[:, bg_col:bg_col + 1], scale=float(s_scale))
                    xo = stg.tile([128, c.GRP * 128], F32, tag="xo")
                    nc.sync.dma_start(xo[:, :gcw], m_in[:, gc0:gc0 + gcw])
                    xn = stg.tile([128, c.GRP * 128], F32, tag="xn")
                    nc.vector.tensor_tensor(
                        out=xn[:, :gcw], in0=t2[:, :gcw], in1=xo[:, :gcw],
                        op=ALU.add)
                    nc.sync.dma_start(m_out[:, gc0:gc0 + gcw], xn[:, :gcw])
                    if mb_out is not None:
                        xb = stg.tile([128, c.GRP * 128], BF16, tag="xb")
                        nc.scalar.activation(xb[:, :gcw], xn[:, :gcw], AF.Copy)
                        nc.sync.dma_start(mb_out[:, gc0:gc0 + gcw], xb[:, :gcw])

        # ---------- layer 1: dense + h', transposes, then per-slab gcn ----------
        with tc.tile_pool(name="hT1", bufs=1) as hTp:
            hT = hTp.tile([128, NS1, c.NLOCP], BF16, tag="hT1")
            with tc.tile_pool(name="xin", bufs=1) as xp, \
                 tc.tile_pool(name="psdense", bufs=2, space="PSUM") as psd, \
                 tc.tile_pool(name="dstg", bufs=3) as dstg:
                half = ((NV + 1023) // 1024) * 512
                xin = xp.tile([128, c.NSLAB_IN, half], BF16, tag="xin")
                for h0 in range(0, NV, half):
                    h1 = min(NV, h0 + half)
                    nc.sync.dma_start(
                        xin[:, :, :h1 - h0],
                        x_t.ap().rearrange("(s p) n -> p s n", p=128)[:, :, h0:h1])
                    dense_pass(xin, h0, h0, h1, "w1", c.NSLAB_IN, NS1,
                               ev_master("b1", m_x1l), psd, dstg)
                    dense_pass(xin, h0, h0, h1, "w1g", c.NSLAB_IN, NS1,
                               ev_h(hT, "b1g"), psd, dstg)
            for s in range(NS1):
                if c.NLOCP > NV:
                    nc.gpsimd.memset(hT[:, s, NV:], 0.0)
                transpose_store(hT, s, s)
        npass = 0
        for s in range(NS1):
            if npass >= c.MAXP:
                break
            last = npass == c.MAXP - 1
            allgather(s)
            gcn_combine(s, m_x1l[s], y_t if last else m_x1[s],
                        None if last else mb_x1[s], bt["bg1"], s, 1.0, True)
            npass += 1

        # ---------- layers 2..6 ----------
        plan = [
            dict(p=2, nin=NS1, xsrc=mb_x1, wdense="w2", bdense="b2",
                 mdense=[m_x2l], wh="w2g", bh="b2g", m_in=m_x2l, m_out=m_x2,
                 mb=mb_x2, bg="bg2", s=1.0, relu=True),
            dict(p=3, nin=1, xsrc=[mb_x2], wdense="w3", bdense="b3",
                 mdense=[m_x3l], wh="w3g", bh="b3g", m_in=m_x3l, m_out=m_x3,
                 mb=mb_x3, bg="bg3", s=0.5, relu=True),
            dict(p=4, nin=1, xsrc=[mb_x3], wdense=None, bdense=None,
                 mdense=None, wh="wg4", bh=None, m_in=m_x3, m_out=m_x4,
                 mb=mb_x4, bg="bg4", s=0.5, relu=True),
            dict(p=5, nin=1, xsrc=[mb_x4], wdense=None, bdense=None,
                 mdense=None, wh="wg5", bh=None, m_in=m_x4, m_out=m_x5,
                 mb=mb_x5, bg="bg5", s=0.25, relu=True),
            dict(p=6, nin=1, xsrc=[mb_x5], wdense=None, bdense=None,
                 mdense=None, wh="wg6", bh=None, m_in=m_x5, m_out=y_t,
                 mb=None, bg="bg6", s=0.25, relu=False),
        ]
        for L in plan:
            if npass >= c.MAXP:
                break
            last = npass == c.MAXP - 1
            if last:
                L["m_out"] = y_t
                L["mb"] = None
            npass += 1
            with tc.tile_pool(name="hTl", bufs=1) as hTp:
                hT = hTp.tile([128, 1, c.NLOCP], BF16, tag="hTl")
                with tc.tile_pool(name="xbf", bufs=1) as xp, \
                     tc.tile_pool(name="psdense", bufs=2, space="PSUM") as psd, \
                     tc.tile_pool(name="dstg", bufs=3) as dstg:
                    xbf = xp.tile([128, L["nin"], NV], BF16, tag="xbf")
                    for si, msrc in enumerate(L["xsrc"]):
                        nc.sync.dma_start(xbf[:, si, :], msrc[:, :])
                    if L["wdense"] is not None:
                        dense_pass(xbf, 0, 0, NV, L["wdense"], L["nin"], 1,
                                   ev_master(L["bdense"], L["mdense"]), psd, dstg)
                    dense_pass(xbf, 0, 0, NV, L["wh"], L["nin"], 1,
                               ev_h(hT, L["bh"]), psd, dstg)
                if c.NLOCP > NV:
                    nc.gpsimd.memset(hT[:, 0, NV:], 0.0)
                transpose_store(hT, 0, L["p"])
            allgather(L["p"])
            gcn_combine(L["p"], L["m_in"], L["m_out"], L["mb"],
                        bt[L["bg"]], 0, L["s"], L["relu"])

    nc.compile()
    return nc


def run(inputs, trace=False):
    cfg = Cfg()
    sched, in_maps = preprocess(cfg, inputs)
    nc = build_nc(cfg, sched)
    res = bass_utils.run_bass_kernel_spmd(
        nc, in_maps, core_ids=list(range(cfg.NC)), trace=trace)
    out = np.empty((cfg.N, cfg.OUT), np.float32)
    for cc in range(cfg.NC):
        out[cc * cfg.NLOC:(cc + 1) * cfg.NLOC] = res.results[cc]["y_t"].T
    return out, res


def kernel(**inputs):
    out, _ = run(inputs)
    return out
